# revision 38
# baseline (speedup 1.0000x reference)
"""ByteBlockTransformerEncoder on 8 NeuronCores (Trainium2, Bass/Tile).

Sharding: data-parallel over batch (B=8 -> one batch element per core),
weights replicated, no collectives. The [S,S] block-diagonal mask is
exploited directly: attention runs per segment (boundaries are known on
host at trace time, so the kernel is specialized to them).

Layouts: hT [E, S] for projections (contraction dim on partitions);
scores computed transposed ST[k, q] per head so the softmax denominator
comes from an appended ones-column in V during the AV matmul; LayerNorm
in [s, e] layout via bn_stats; residuals injected into PSUM with
identity matmuls. q/k live in a padded layout (3 heads per 128
partitions at 32-row offsets) because PE operands must start at
partition 0/32/64.
"""

import sys

import numpy as np

if "/opt/trn_rl_repo" not in sys.path:
    sys.path.insert(0, "/opt/trn_rl_repo")

import concourse.bass as bass
import concourse.tile as tile
from concourse import mybir
from concourse.bass import ts
from concourse.masks import make_identity

B, S, E, H, F, L, V, DH = 8, 2048, 128, 8, 512, 4, 256, 16
NT = S // 128  # 16 s-tiles
NC4 = S // 512  # 4 512-chunks
QCH = 128  # q-tile width inside a segment
KGRP = 4  # k-tiles per scores/exp group (bounds PT SBUF footprint)
EPS = 1e-5
SCALE = 1.0 / float(np.sqrt(DH))
f32 = mybir.dt.float32
AF = mybir.ActivationFunctionType
OP = mybir.AluOpType


def _segments(boundaries):
    bs = sorted(int(b) for b in np.asarray(boundaries).reshape(-1))
    edges = [0] + [min(max(b, 0), S) for b in bs] + [S]
    segs = []
    for i in range(len(edges) - 1):
        if edges[i + 1] > edges[i]:
            segs.append((edges[i], edges[i + 1] - edges[i]))
    return segs


def _r32(n):
    return min((n + 31) // 32 * 32, 128)


def _tiles(start, length, width):
    out = []
    k = start
    while k < start + length:
        out.append((k, min(width, start + length - k)))
        k += width
    return out


def split_multiwaits(nc, drain_max=1, other_max=1):
    """Walrus codegen rejects instructions carrying more sem waits than the
    ISA struct allows (1 for CTRL/Drain, ~2 for compute). Hoist excess waits
    onto same-engine NoOps inserted right before (earlier waits on the same
    engine preserve semantics)."""
    for f in nc.m.functions:
        for bb in f.blocks:
            il = bb.instructions
            i = 0
            while i < len(il):
                ins = il[i]
                si = ins.sync_info
                max_waits = drain_max if type(ins).__name__ == "InstDrain" else other_max
                if si is not None and si.on_wait and len(si.on_wait) > max_waits:
                    waits = list(si.on_wait)
                    si.on_wait = waits[:max_waits]
                    ins.sync_info = si
                    rest = waits[max_waits:]
                    pre = []
                    for j in range(0, len(rest), max_waits):
                        nop = nc.engines[ins.engine].nop(nofuse=True).ins
                        for f2 in nc.m.functions:
                            for bb2 in f2.blocks:
                                il2 = bb2.instructions
                                if il2 and il2[-1].name == nop.name:
                                    il2.pop()
                        nsi = nop.sync_info
                        if nsi is None:
                            nsi = mybir.SyncInfo(
                                on_wait=rest[j : j + max_waits], on_update=[]
                            )
                        else:
                            nsi.on_wait = rest[j : j + max_waits]
                        nop.sync_info = nsi
                        pre.append(nop)
                    il[i:i] = pre
                    i += len(pre)
                i += 1


def build(nc, segs, use_bias, repeat=1, debug_mode=None):
    dt = f32
    x_f = nc.dram_tensor("xf", [1, S], f32, kind="ExternalInput").ap()
    embt_d = nc.dram_tensor("embt", [128, 2 * E], f32, kind="ExternalInput").ap()
    iota_d = nc.dram_tensor("iota", [128, 2], f32, kind="ExternalInput").ap()
    wv_d = nc.dram_tensor("wvT", [128, L, E], f32, kind="ExternalInput").ap()
    # per-head q/k projection weights; [e, L, (q|k), h, d]
    wqkh_d = nc.dram_tensor("wqkh", [128, L, 2, H, DH], f32, kind="ExternalInput").ap()
    woT_d = nc.dram_tensor("woT", [128, L, E], f32, kind="ExternalInput").ap()
    w1T_d = nc.dram_tensor("w1T", [128, L, F], f32, kind="ExternalInput").ap()
    w2T_d = nc.dram_tensor("w2T", [128, L, 4, E], f32, kind="ExternalInput").ap()
    out_d = nc.dram_tensor("out", [128, 1], f32, kind="ExternalOutput").ap()
    bias_d = {}
    if use_bias["bqkv"]:
        bias_d["bqk_c"] = nc.dram_tensor("bqk_c", [128, L, 2], f32, kind="ExternalInput").ap()
        bias_d["bqkv_v"] = nc.dram_tensor("bqkv_v", [1, L, E], f32, kind="ExternalInput").ap()
    if use_bias["bo"]:
        bias_d["bo_r"] = nc.dram_tensor("bo_r", [1, L, E], f32, kind="ExternalInput").ap()
    if use_bias["b1"]:
        bias_d["b1_c"] = nc.dram_tensor("b1_c", [128, L, 4], f32, kind="ExternalInput").ap()
    if use_bias["b2"]:
        bias_d["b2_r"] = nc.dram_tensor("b2_r", [1, L, E], f32, kind="ExternalInput").ap()
    if use_bias["ln"]:
        bias_d["ln_gb"] = nc.dram_tensor(
            "ln_gb", [1, L, 2, 2, E], f32, kind="ExternalInput"
        ).ap()  # [l, which_ln, g|b, e]

    # attention plan per segment: (ktiles, qtiles)
    seg_plans = [(_tiles(s0, ln, 128), _tiles(s0, ln, QCH)) for s0, ln in segs]

    with tile.TileContext(nc) as tc:
        with (
            tc.tile_pool(name="const", bufs=1) as constp,
            tc.tile_pool(name="state", bufs=1) as statep,
            tc.tile_pool(name="wts", bufs=2) as wtsp,
            tc.tile_pool(name="p512", bufs=2, space="PSUM") as p512,
            tc.tile_pool(name="p128", bufs=2, space="PSUM") as p128,
            tc.tile_pool(name="st", bufs=2, space="PSUM") as stp,
            tc.tile_pool(name="pt", bufs=KGRP + 2) as ptp,
            tc.tile_pool(name="vaug", bufs=1) as vaugp,
            tc.tile_pool(name="small", bufs=8) as smallp,
            tc.tile_pool(name="oseg", bufs=3) as osegp,
            tc.tile_pool(name="ffn", bufs=2) as ffnp,
            tc.tile_pool(name="qk", bufs=1) as qkp,
        ):
            # ---- constants ----
            ident = constp.tile([128, 128], dt)
            make_identity(nc, ident)
            ones_col = constp.tile([128, 1], dt)
            nc.vector.memset(ones_col, 1.0)
            eps_col = constp.tile([128, 1], f32)
            nc.vector.memset(eps_col, EPS)
            iota_sb = constp.tile([128, 2], f32)
            nc.sync.dma_start(out=iota_sb, in_=iota_d)
            embt = constp.tile([128, 2, E], dt)
            nc.sync.dma_start(out=embt, in_=embt_d.rearrange("p (t e) -> p t e", t=2))
            bias_sb = {}
            for name, d in bias_d.items():
                if name in ("bqkv_v", "bo_r", "b2_r", "ln_gb"):
                    sh = [128] + list(d.shape[1:])
                    t = constp.tile(sh, f32, name=name)
                    nc.sync.dma_start(out=t, in_=d.to_broadcast(sh))
                else:
                    t = constp.tile(list(d.shape), f32, name=name)
                    nc.sync.dma_start(out=t, in_=d)
                bias_sb[name] = t
            ones_row = None
            if use_bias["bqkv"] or use_bias["bo"] or use_bias["b2"]:
                ones_row = constp.tile([1, 128], dt)
                nc.vector.memset(ones_row, 1.0)

            # head-interleaved q/k tiles: [d(32 rows, 16-31 zero), h, s]
            bf16 = mybir.dt.bfloat16
            qT2 = statep.tile([32, H, S + 32], bf16, tag="qT2")
            kT2 = statep.tile([32, H, S + 32], bf16, tag="kT2")
            nc.vector.memset(qT2, 0.0)
            nc.vector.memset(kT2, 0.0)

            # ---- embedding via one-hot matmul (chunked) ----
            hT = statep.tile([128, S + 32], dt, tag="hT")
            nc.vector.memset(hT[:, S : S + 32], 0.0)
            h = statep.tile([128, NT, E], dt, tag="h")
            with tc.tile_pool(name="emb_tmp", bufs=2) as embp:
                for c in range(NC4):
                    xbc = embp.tile([128, 512], f32, tag="xbc")
                    nc.sync.dma_start(
                        out=xbc, in_=x_f[:, ts(c, 512)].to_broadcast([128, 512])
                    )
                    oh = embp.tile([128, 2, 512], dt, tag="oh")
                    for vt in range(2):
                        nc.vector.tensor_scalar(
                            out=oh[:, vt, :],
                            in0=xbc,
                            scalar1=iota_sb[:, vt : vt + 1],
                            scalar2=None,
                            op0=OP.is_equal,
                        )
                    ps = p512.tile([128, 512], f32, tag="mm512")
                    for vt in range(2):
                        nc.tensor.matmul(
                            ps,
                            lhsT=embt[:, vt, :],
                            rhs=oh[:, vt, :],
                            start=(vt == 0),
                            stop=(vt == 1),
                        )
                    nc.vector.tensor_copy(out=hT[:, ts(c, 512)], in_=ps)
                for t in range(NT):
                    tp = p128.tile([128, 128], f32, tag="mm128")
                    nc.tensor.transpose(tp, hT[:, ts(t, 128)], ident)
                    nc.vector.tensor_copy(out=h[:, t, :], in_=tp)

            # ---- layers ----
            layers = list(range(L)) * repeat
            for li, l in enumerate(layers):
                is_last = li == len(layers) - 1
                # per-layer weights
                wv_l = wtsp.tile([128, E], dt, tag="wv")
                nc.sync.dma_start(out=wv_l, in_=wv_d[:, l, :])
                wqkh_l = wtsp.tile([128, 2, H, DH], dt, tag="wqkh")
                nc.sync.dma_start(out=wqkh_l, in_=wqkh_d[:, l, :, :, :])
                woT_l = wtsp.tile([128, E], dt, tag="wo")
                nc.sync.dma_start(out=woT_l, in_=woT_d[:, l, :])
                w1T_l = wtsp.tile([128, F], dt, tag="w1")
                nc.sync.dma_start(out=w1T_l, in_=w1T_d[:, l, :])
                w2T_l = wtsp.tile([128, 4, E], dt, tag="w2")
                nc.sync.dma_start(out=w2T_l, in_=w2T_d[:, l, :, :])

                # q/k projections: standard full-width matmul into a bf16
                # temp, then per-head DMA rearrange into the head-interleaved
                # base-0 tiles (DMA has no partition-alignment limits)
                qk_tmp = qkp.tile([128, 2, S], bf16, tag="qk_tmp")
                for c in range(NC4):
                    for qk in range(2):
                        ps = p512.tile([128, 512], f32, tag="mm512")
                        nc.tensor.matmul(
                            ps,
                            lhsT=wqkh_l[:, qk, :, :].rearrange("e h d -> e (h d)"),
                            rhs=hT[:, ts(c, 512)],
                            start=True,
                            stop=True,
                        )
                        bias = 0.0
                        if use_bias["bqkv"]:
                            bias = bias_sb["bqk_c"][:, l, qk : qk + 1]
                        nc.scalar.activation(
                            out=qk_tmp[:, qk, ts(c, 512)],
                            in_=ps,
                            func=AF.Copy,
                            bias=bias,
                        )
                for qk, dst in ((0, qT2), (1, kT2)):
                    for hh in range(H):
                        nc.sync.dma_start(
                            out=dst[0:16, hh, 0:S],
                            in_=qk_tmp[16 * hh : 16 * hh + 16, qk, :],
                        )

                # v per segment k-tile, augmented with ones column
                vaug_tiles = {}
                for si, (ktiles, qtiles) in enumerate(seg_plans):
                    for ki, (ks, kl) in enumerate(ktiles):
                        kl32 = _r32(kl)
                        vps = p128.tile([128, 128], f32, tag="mm128")
                        nc.tensor.matmul(
                            vps[:kl32, :],
                            lhsT=hT[:, ks : ks + kl32],
                            rhs=wv_l,
                            start=True,
                            stop=not use_bias["bqkv"],
                        )
                        if use_bias["bqkv"]:
                            nc.tensor.matmul(
                                vps[:kl32, :],
                                lhsT=ones_row[:, :kl32],
                                rhs=bias_sb["bqkv_v"][0:1, l, :],
                                start=False,
                                stop=True,
                            )
                        va = vaugp.tile(
                            [128, H, 17], dt, tag=f"vaug_{ks}", name=f"vaug_{ks}"
                        )
                        nc.vector.memset(va, 0.0)
                        nc.vector.tensor_copy(
                            out=va[:kl, :, 0:16],
                            in_=vps[:kl, :].rearrange("k (h d) -> k h d", h=H),
                        )
                        nc.vector.memset(va[:kl, :, 16:17], 1.0)
                        vaug_tiles[(si, ki)] = va

                # attention per segment -> oT [e, s]
                oT = statep.tile([128, S], dt, tag="oT")
                if debug_mode == "noattn":
                    for c in range(NC4):
                        nc.vector.tensor_copy(out=oT[:, ts(c, 512)], in_=hT[:, ts(c, 512)])
                for si, (ktiles, qtiles) in enumerate(seg_plans if debug_mode != "noattn" else []):
                    nkt = len(ktiles)
                    for qs, qn in qtiles:
                        qn32 = _r32(qn)
                        oa = p128.tile([128, H, 17], f32, tag="mm128")
                        for g0 in range(0, nkt, KGRP):
                            pts = []
                            for ki in range(g0, min(g0 + KGRP, nkt)):
                                ks, kl = ktiles[ki]
                                kl32 = _r32(kl)
                                pt = ptp.tile([128, H, QCH], dt, tag="pt")
                                if debug_mode == "noscores":
                                    nc.vector.memset(pt, 1.0)
                                elif debug_mode == "noexp2":
                                    st = stp.tile([128, H, QCH], f32, tag="st")
                                    for hh in range(H):
                                        nc.tensor.matmul(
                                            st[:kl32, hh, :qn32],
                                            lhsT=kT2[:, hh, ks : ks + kl32],
                                            rhs=qT2[:, hh, qs : qs + qn32],
                                            start=True,
                                            stop=True,
                                        )
                                    nc.vector.memset(pt, 1.0)
                                else:
                                    st = stp.tile([128, H, QCH], f32, tag="st")
                                    for hh in range(H):
                                        nc.tensor.matmul(
                                            st[:kl32, hh, :qn32],
                                            lhsT=kT2[:, hh, ks : ks + kl32],
                                            rhs=qT2[:, hh, qs : qs + qn32],
                                            start=True,
                                            stop=True,
                                        )
                                    nc.scalar.activation(
                                        out=pt[:kl32, :, :qn32],
                                        in_=st[:kl32, :, :qn32],
                                        func=AF.Copy if debug_mode == "noexp" else AF.Exp,
                                        scale=SCALE,
                                    )
                                pts.append((ki, kl32, pt))
                            if debug_mode == "noav":
                                if g0 == 0:
                                    nc.vector.memset(oa, 1.0)
                            else:
                                for hh in range(H):
                                    for ki, kl32, pt in pts:
                                        nc.tensor.matmul(
                                            oa[:qn32, hh, :],
                                            lhsT=pt[:kl32, hh, :qn32],
                                            rhs=vaug_tiles[(si, ki)][:kl32, hh, :],
                                            start=(ki == 0),
                                            stop=(ki == nkt - 1),
                                        )
                        rec = smallp.tile([128, H], f32, tag="rec")
                        nc.vector.reciprocal(out=rec[:qn32, :], in_=oa[:qn32, :, 16])
                        oseg = osegp.tile([128, E], dt, tag="oseg")
                        nc.vector.tensor_tensor(
                            out=oseg[:qn32, :].rearrange("q (h d) -> q h d", h=H),
                            in0=oa[:qn32, :, 0:16],
                            in1=rec[:qn32, :, None].to_broadcast([qn32, H, 16]),
                            op=OP.mult,
                        )
                        tp = p128.tile([128, 128], f32, tag="mm128")
                        nc.tensor.transpose(tp[:, :qn32], oseg[:qn32, :], ident[:qn32, :qn32])
                        nc.vector.tensor_copy(out=oT[:, qs : qs + qn], in_=tp[:, :qn])

                # z = h + oT.T @ Wo^T (+bo); LN1 in-place: r1 -> h1
                r1 = statep.tile([128, NT, E], dt, tag="r1")
                mv1 = smallp.tile([128, NT, 2], f32, tag="mv1")
                for t in range(NT):
                    zps = p128.tile([128, 128], f32, tag="mm128")
                    nc.tensor.matmul(
                        zps, lhsT=hT[:, ts(t, 128)], rhs=ident, start=True, stop=False
                    )
                    nc.tensor.matmul(
                        zps,
                        lhsT=oT[:, ts(t, 128)],
                        rhs=woT_l,
                        start=False,
                        stop=not use_bias["bo"],
                    )
                    if use_bias["bo"]:
                        nc.tensor.matmul(
                            zps,
                            lhsT=ones_row,
                            rhs=bias_sb["bo_r"][0:1, l, :],
                            start=False,
                            stop=True,
                        )
                    nc.vector.tensor_copy(out=r1[:, t, :], in_=zps)
                    stats = smallp.tile([128, 6], f32, tag="stats")
                    nc.vector.bn_stats(out=stats, in_=r1[:, t, :])
                    nc.vector.bn_aggr(out=mv1[:, t, :], in_=stats)
                rstd1 = smallp.tile([128, NT], f32, tag="rstd1")
                nc.scalar.activation(out=rstd1, in_=mv1[:, :, 1], func=AF.Sqrt, bias=eps_col)
                nc.vector.reciprocal(out=rstd1, in_=rstd1)
                h1 = r1
                h1T = statep.tile([128, S], dt, tag="h1T")
                for t in range(NT):
                    nc.vector.tensor_scalar(
                        out=h1[:, t, :],
                        in0=r1[:, t, :],
                        scalar1=mv1[:, t, 0:1],
                        scalar2=rstd1[:, t : t + 1],
                        op0=OP.subtract,
                        op1=OP.mult,
                    )
                    if use_bias["ln"]:
                        nc.vector.tensor_mul(
                            out=h1[:, t, :], in0=h1[:, t, :],
                            in1=bias_sb["ln_gb"][:, l, 0, 0, :],
                        )
                        nc.vector.tensor_add(
                            out=h1[:, t, :], in0=h1[:, t, :],
                            in1=bias_sb["ln_gb"][:, l, 0, 1, :],
                        )
                    tp = p128.tile([128, 128], f32, tag="mm128")
                    nc.tensor.transpose(tp, h1[:, t, :], ident)
                    nc.vector.tensor_copy(out=h1T[:, ts(t, 128)], in_=tp)

                # FFN (s-chunked): fT = relu(W1 @ h1T + b1); y = h1 + fT.T @ W2^T
                r2 = statep.tile([128, NT, E], dt, tag="r2")
                mv2 = smallp.tile([128, NT, 2], f32, tag="mv2")
                for c in range(NC4):
                    fTc = ffnp.tile([128, 4, 512], dt, tag="fTc")
                    for jt in range(4):
                        fps = p512.tile([128, 512], f32, tag="mm512")
                        nc.tensor.matmul(
                            fps,
                            lhsT=w1T_l[:, ts(jt, 128)],
                            rhs=h1T[:, ts(c, 512)],
                            start=True,
                            stop=True,
                        )
                        if use_bias["b1"]:
                            nc.vector.tensor_scalar(
                                out=fTc[:, jt, :],
                                in0=fps,
                                scalar1=bias_sb["b1_c"][:, l, jt : jt + 1],
                                scalar2=0.0,
                                op0=OP.add,
                                op1=OP.max,
                            )
                        else:
                            nc.vector.tensor_scalar(
                                out=fTc[:, jt, :],
                                in0=fps,
                                scalar1=0.0,
                                scalar2=None,
                                op0=OP.max,
                            )
                    for tt in range(4):
                        t = 4 * c + tt
                        yps = p128.tile([128, 128], f32, tag="mm128")
                        nc.tensor.matmul(
                            yps, lhsT=h1T[:, ts(t, 128)], rhs=ident,
                            start=True, stop=False,
                        )
                        for ft in range(4):
                            nc.tensor.matmul(
                                yps,
                                lhsT=fTc[:, ft, ts(tt, 128)],
                                rhs=w2T_l[:, ft, :],
                                start=False,
                                stop=(ft == 3) and not use_bias["b2"],
                            )
                        if use_bias["b2"]:
                            nc.tensor.matmul(
                                yps,
                                lhsT=ones_row,
                                rhs=bias_sb["b2_r"][0:1, l, :],
                                start=False,
                                stop=True,
                            )
                        nc.vector.tensor_copy(out=r2[:, t, :], in_=yps)
                        stats = smallp.tile([128, 6], f32, tag="stats")
                        nc.vector.bn_stats(out=stats, in_=r2[:, t, :])
                        nc.vector.bn_aggr(out=mv2[:, t, :], in_=stats)
                rstd2 = smallp.tile([128, NT], f32, tag="rstd2")
                nc.scalar.activation(out=rstd2, in_=mv2[:, :, 1], func=AF.Sqrt, bias=eps_col)
                nc.vector.reciprocal(out=rstd2, in_=rstd2)
                h = r2
                if not is_last:
                    hT = statep.tile([128, S + 32], dt, tag="hT")
                    nc.vector.memset(hT[:, S : S + 32], 0.0)
                for t in range(NT):
                    nc.vector.tensor_scalar(
                        out=h[:, t, :],
                        in0=r2[:, t, :],
                        scalar1=mv2[:, t, 0:1],
                        scalar2=rstd2[:, t : t + 1],
                        op0=OP.subtract,
                        op1=OP.mult,
                    )
                    if use_bias["ln"]:
                        nc.vector.tensor_mul(
                            out=h[:, t, :], in0=h[:, t, :],
                            in1=bias_sb["ln_gb"][:, l, 1, 0, :],
                        )
                        nc.vector.tensor_add(
                            out=h[:, t, :], in0=h[:, t, :],
                            in1=bias_sb["ln_gb"][:, l, 1, 1, :],
                        )
                    if not is_last:
                        tp = p128.tile([128, 128], f32, tag="mm128")
                        nc.tensor.transpose(tp, h[:, t, :], ident)
                        nc.vector.tensor_copy(out=hT[:, ts(t, 128)], in_=tp)

            # ---- mean pool over s ----
            acc = p128.tile([128, 1], f32, tag="mm128")
            for t in range(NT):
                nc.tensor.matmul(
                    acc,
                    lhsT=h[:, t, :],
                    rhs=ones_col,
                    start=(t == 0),
                    stop=(t == NT - 1),
                )
            out_sb = smallp.tile([128, 1], f32, tag="out")
            nc.scalar.mul(out=out_sb, in_=acc, mul=1.0 / S)
            nc.sync.dma_start(out=out_d, in_=out_sb)

    split_multiwaits(nc)
    return nc


def _prep(x, boundaries, emb, Wqkv, bqkv, Wo, bo, W1, b1, W2, b2,
          ln1_g, ln1_b, ln2_g, ln2_b):
    x = np.asarray(x)
    emb = np.asarray(emb, np.float32)
    Wqkv = np.asarray(Wqkv, np.float32)
    Wo = np.asarray(Wo, np.float32)
    W1 = np.asarray(W1, np.float32)
    W2 = np.asarray(W2, np.float32)
    bqkv = np.asarray(bqkv, np.float32)
    bo = np.asarray(bo, np.float32)
    b1 = np.asarray(b1, np.float32)
    b2 = np.asarray(b2, np.float32)
    ln1_g = np.asarray(ln1_g, np.float32)
    ln1_b = np.asarray(ln1_b, np.float32)
    ln2_g = np.asarray(ln2_g, np.float32)
    ln2_b = np.asarray(ln2_b, np.float32)

    segs = _segments(boundaries)
    use_bias = {
        "bqkv": bool(np.any(bqkv != 0)),
        "bo": bool(np.any(bo != 0)),
        "b1": bool(np.any(b1 != 0)),
        "b2": bool(np.any(b2 != 0)),
        "ln": bool(
            np.any(ln1_g != 1) or np.any(ln1_b != 0)
            or np.any(ln2_g != 1) or np.any(ln2_b != 0)
        ),
    }

    # per-head q/k projection weights [e, L, qk, h, d]
    wqkh = (
        Wqkv[:, : 2 * E, :].reshape(L, 2, H, DH, E).transpose(4, 0, 1, 2, 3).copy()
    )

    shared = {
        "embt": emb.reshape(2, 128, E).transpose(1, 0, 2).reshape(128, 2 * E).copy(),
        "iota": np.arange(V, dtype=np.float32).reshape(2, 128).T.copy(),
        "wvT": Wqkv[:, 2 * E : 3 * E, :].transpose(2, 0, 1).copy(),  # [e, L, E]
        "wqkh": wqkh,
        "woT": Wo.transpose(2, 0, 1).copy(),  # [e, L, E]
        "w1T": W1.transpose(2, 0, 1).copy(),  # [e, L, F]
        "w2T": W2.transpose(0, 2, 1).reshape(L, 4, 128, E).transpose(2, 0, 1, 3).copy(),
    }
    if use_bias["bqkv"]:
        shared["bqk_c"] = bqkv[:, : 2 * E].reshape(L, 2, 128).transpose(2, 0, 1).copy()
        shared["bqkv_v"] = bqkv[:, 2 * E : 3 * E].reshape(1, L, E).copy()
    if use_bias["bo"]:
        shared["bo_r"] = bo.reshape(1, L, E).copy()
    if use_bias["b1"]:
        shared["b1_c"] = b1.reshape(L, 4, 128).transpose(2, 0, 1).copy()
    if use_bias["b2"]:
        shared["b2_r"] = b2.reshape(1, L, E).copy()
    if use_bias["ln"]:
        ln_gb = np.stack(
            [np.stack([ln1_g, ln1_b], 1), np.stack([ln2_g, ln2_b], 1)], 1
        )  # [L, 2, 2, E]
        shared["ln_gb"] = ln_gb.reshape(1, L, 2, 2, E).copy()

    xf = [x[b].astype(np.float32).reshape(1, S) for b in range(B)]
    return segs, use_bias, shared, xf


def build_from_inputs(repeat=1, debug_mode=None, **inputs):
    segs, use_bias, shared, xf = _prep(**inputs)
    nc = bass.Bass()
    build(nc, segs, use_bias, repeat=repeat, debug_mode=debug_mode)
    in_maps = [dict(shared, xf=xf[b]) for b in range(B)]
    return nc, in_maps


def kernel(**inputs):
    from concourse.bass_utils import run_bass_kernel_spmd

    nc, in_maps = build_from_inputs(**inputs)
    res = run_bass_kernel_spmd(nc, in_maps, core_ids=list(range(B)))
    out = np.stack([res.results[b]["out"].reshape(E) for b in range(B)])
    return out.astype(np.float32)

